# revision 1
# baseline (speedup 1.0000x reference)
"""Trainium2 Bass kernel for nn_CrossImageAttentionLayer.

Contract: kernel(**inputs) takes FULL unsharded inputs (as produced by
setup_inputs) and returns the FULL (B, N, D) output. Internally shards the
flattened (B*N) query dimension across 8 NeuronCores (all-reduce-free),
builds one SPMD Bass/Tile kernel, and runs it via run_bass_kernel_spmd.

Algorithm per core (4096 queries of one batch b):
  setup (on device):
    A   = diag(gamma) @ Wq @ Wk^T          ->  qt = rs*(xc@A) + c_qt
    c_qt = (beta@Wq + bq) @ Wk^T
    W_vo = Wv @ Wo, b_voWo = bv@Wo, a_qbk = gamma*(Wq@bk), c_qbk = beta@(Wq@bk)+bq.bk
  per tile of 128 queries:
    layernorm stats; xc^T via PE transpose; qt = xc@A (PE) scaled by rs (ACT)
    bilinear indices/weights from pixel_coords (PE transpose + small vector ops)
    ONE indirect DMA gathers 16 rows x 512 floats per query (8 cams x 2 y-rows,
      x-pair adjacency makes each row a contiguous 2KB run)
    scale gathered corners by bilinear weights (split across DVE/ACT/POOL)
    corner-sum -> s_c; scores via fused tensor_tensor_reduce dots
    masked softmax over 8 cameras (all-invalid -> 0)
    g = sum_c attn_c*s_c (+ sum_attn "ones feature" for the bv term)
    out = x + g@W_vo + sum_attn*(bv@Wo) + bo   (PE, biases as K=1 matmuls)
"""
import math
import numpy as np
from contextlib import ExitStack

import concourse.bass as bass
import concourse.tile as tile
import concourse.mybir as mybir
from concourse.bass import AP
from concourse.masks import make_identity

F32 = mybir.dt.float32
BF16 = mybir.dt.bfloat16
I32 = mybir.dt.int32
ALU = mybir.AluOpType
ACTF = mybir.ActivationFunctionType

# problem constants (hardcoded per harness contract)
B, C, N, D, H, W = 2, 8, 16384, 256, 64, 64
NCORES = 8
ROWS_PER_CORE = (B * N) // NCORES      # 4096
P = 128                                # tile = 128 queries
EPS = 1e-5
SCALE = 1.0 / math.sqrt(float(D))

USE_BF16 = True                        # compute dtype for sampled features


def build_nc(rows_per_core=ROWS_PER_CORE, use_bf16=USE_BF16, debug=False):
    DT = BF16 if use_bf16 else F32
    ntiles = rows_per_core // P
    nc = bass.Bass()

    x_d = nc.dram_tensor("x", (rows_per_core, D), F32, kind="ExternalInput")
    feat_d = nc.dram_tensor("feat", (C * H * W, D), F32, kind="ExternalInput")
    coords_d = nc.dram_tensor("coords", (C, rows_per_core, 2), F32,
                              kind="ExternalInput")
    valid_d = nc.dram_tensor("valid", (C, rows_per_core), I32,
                             kind="ExternalInput")
    wq_d = nc.dram_tensor("Wq", (D, D), F32, kind="ExternalInput")
    bq_d = nc.dram_tensor("bq", (D,), F32, kind="ExternalInput")
    wkv_d = nc.dram_tensor("Wkv", (D, 2 * D), F32, kind="ExternalInput")
    bkv_d = nc.dram_tensor("bkv", (2 * D,), F32, kind="ExternalInput")
    wo_d = nc.dram_tensor("Wo", (D, D), F32, kind="ExternalInput")
    bo_d = nc.dram_tensor("bo", (D,), F32, kind="ExternalInput")
    gamma_d = nc.dram_tensor("gamma", (D,), F32, kind="ExternalInput")
    beta_d = nc.dram_tensor("beta", (D,), F32, kind="ExternalInput")
    out_d = nc.dram_tensor("out", (rows_per_core, D), F32,
                           kind="ExternalOutput")
    dbg = {}
    if debug:
        dbg["qt"] = nc.dram_tensor("dbg_qt", (rows_per_core, D), F32,
                                   kind="ExternalOutput")
        dbg["scores"] = nc.dram_tensor("dbg_scores", (rows_per_core, C), F32,
                                       kind="ExternalOutput")
        dbg["attn"] = nc.dram_tensor("dbg_attn", (rows_per_core, C), F32,
                                     kind="ExternalOutput")
        dbg["s0"] = nc.dram_tensor("dbg_s0", (rows_per_core, D), F32,
                                   kind="ExternalOutput")
        dbg["g"] = nc.dram_tensor("dbg_g", (rows_per_core, D), F32,
                                  kind="ExternalOutput")
        dbg["A"] = nc.dram_tensor("dbg_A", (2, P, D), F32,
                                  kind="ExternalOutput")
        dbg["cqt"] = nc.dram_tensor("dbg_cqt", (1, D), F32,
                                    kind="ExternalOutput")
        dbg["xcT"] = nc.dram_tensor("dbg_xcT", (rows_per_core // P, P, 2 * P),
                                    F32, kind="ExternalOutput")
        dbg["qtps"] = nc.dram_tensor("dbg_qtps", (rows_per_core, D), F32,
                                     kind="ExternalOutput")
        dbg["mv"] = nc.dram_tensor("dbg_mv", (rows_per_core, 2), F32,
                                   kind="ExternalOutput")
        dbg["rs"] = nc.dram_tensor("dbg_rs", (rows_per_core, 1), F32,
                                   kind="ExternalOutput")
        dbg["offs"] = nc.dram_tensor("dbg_offs", (rows_per_core, 2 * C), I32,
                                     kind="ExternalOutput")
        dbg["wgt"] = nc.dram_tensor("dbg_wgt", (rows_per_core, 4 * C), F32,
                                    kind="ExternalOutput")
        dbg["tg0"] = nc.dram_tensor("dbg_tg0", (P, 2, 2, D), F32,
                                    kind="ExternalOutput")
        dbg["camb"] = nc.dram_tensor("dbg_camb", (P, C), I32,
                                     kind="ExternalOutput")

    with tile.TileContext(nc) as tc, ExitStack() as ctx:
        const = ctx.enter_context(tc.tile_pool(name="const", bufs=1))

        # ---------- constants ----------
        ident = const.tile([P, P], F32)
        make_identity(nc, ident[:])
        ones_row = const.tile([1, D], F32)
        nc.vector.memset(ones_row[:], 1.0)
        b315 = const.tile([P, 1], F32)
        nc.vector.memset(b315[:], 31.5)
        epsb = const.tile([P, 1], F32)
        nc.vector.memset(epsb[:], EPS)
        ones8 = const.tile([P, C], F32)
        nc.vector.memset(ones8[:], 1.0)
        cam_base = const.tile([P, C], I32)
        nc.gpsimd.iota(cam_base[:], pattern=[[H * W, C]], base=0,
                       channel_multiplier=0)

        # ---------- persistent weight products ----------
        A_s = const.tile([P, 2, D], F32)       # diag(gamma) Wq Wk^T, rows on part
        Wvo_s = const.tile([P, 2, D], F32)     # Wv @ Wo
        cqt_row = const.tile([1, D], F32)
        aqbkT = const.tile([P, 2], F32)        # (gamma*Wq@bk) as column slabs
        bvoWo_row = const.tile([1, D], F32)
        bo_row = const.tile([1, D], F32)
        cqbk16_b = const.tile([P, 1], F32)     # (beta@Wq@bk + bq.bk)/sqrt(D) bcast

        with tc.tile_pool(name="setup", bufs=1) as sp, \
             tc.tile_pool(name="setup_ps", bufs=1, space="PSUM") as spp:
            wq_s = sp.tile([P, 2, D], F32)
            nc.sync.dma_start(wq_s[:], wq_d.rearrange("(j p) d -> p j d", p=P))
            wkv_s = sp.tile([P, 2, 2 * D], F32)
            nc.sync.dma_start(wkv_s[:], wkv_d.rearrange("(j p) d -> p j d", p=P))
            wo_s = sp.tile([P, 2, D], F32)
            nc.sync.dma_start(wo_s[:], wo_d.rearrange("(j p) d -> p j d", p=P))
            nc.sync.dma_start(bo_row[:], bo_d[None, :])
            gam_row = sp.tile([1, D], F32)
            nc.sync.dma_start(gam_row[:], gamma_d[None, :])
            bet_row = sp.tile([1, D], F32)
            nc.sync.dma_start(bet_row[:], beta_d[None, :])
            bq_row = sp.tile([1, D], F32)
            nc.sync.dma_start(bq_row[:], bq_d[None, :])
            bkv_row = sp.tile([1, 2 * D], F32)
            nc.sync.dma_start(bkv_row[:], bkv_d[None, :])

            def transpose_256(dst, src):
                # dst[p, jj, :] = src^T slabs; src given as [P, 2, D] tile view
                for jj in range(2):
                    pst = spp.tile([P, P], F32, tag="tp")
                    for i in range(2):
                        nc.tensor.transpose(
                            pst[:], src[:, i, jj * P:(jj + 1) * P], ident[:])
                        nc.scalar.copy(dst[:, jj, i * P:(i + 1) * P], pst[:])
                        if i == 0:
                            pst = spp.tile([P, P], F32, tag="tp")

            wqT = sp.tile([P, 2, D], F32)
            transpose_256(wqT, wq_s)
            wkT = sp.tile([P, 2, D], F32)
            transpose_256(wkT, wkv_s[:, :, 0:D])
            wvT = sp.tile([P, 2, D], F32)
            transpose_256(wvT, wkv_s[:, :, D:2 * D])

            def col_of(row_ap, tag):
                # [1, 128] row -> [128, 1] column (K=1 transpose)
                ps = spp.tile([P, 1], F32, tag="col")
                nc.tensor.transpose(ps[:], row_ap, ident[0:1, 0:1])
                sb = sp.tile([P, 1], F32, tag=tag)
                nc.scalar.copy(sb[:], ps[:])
                return sb

            gcol = [col_of(gam_row[0:1, k * P:(k + 1) * P], f"gcol{k}")
                    for k in range(2)]
            betcol = [col_of(bet_row[0:1, k * P:(k + 1) * P], f"betcol{k}")
                      for k in range(2)]
            bkcol = [col_of(bkv_row[0:1, k * P:(k + 1) * P], f"bkcol{k}")
                     for k in range(2)]
            bvcol = [col_of(bkv_row[0:1, D + k * P:D + (k + 1) * P],
                            f"bvcol{k}") for k in range(2)]

            # A = Wq @ Wk^T (M-blocks i), then scale rows by gamma into A_s
            for i in range(2):
                psA = spp.tile([P, D], F32, tag="mm")
                for k in range(2):
                    nc.tensor.matmul(psA[:], wqT[:, k, i * P:(i + 1) * P],
                                     wkT[:, k, :], start=(k == 0),
                                     stop=(k == 1))
                nc.vector.tensor_scalar(A_s[:, i, :], psA[:], gcol[i][:],
                                        None, ALU.mult)

            # W_vo = Wv @ Wo
            for i in range(2):
                psV = spp.tile([P, D], F32, tag="mm")
                for k in range(2):
                    nc.tensor.matmul(psV[:], wvT[:, k, i * P:(i + 1) * P],
                                     wo_s[:, k, :], start=(k == 0),
                                     stop=(k == 1))
                nc.scalar.copy(Wvo_s[:, i, :], psV[:])

            # u = beta@Wq + bq  (row)
            psu = spp.tile([1, D], F32, tag="row")
            for k in range(2):
                nc.tensor.matmul(psu[:], betcol[k][:], wq_s[:, k, :],
                                 start=(k == 0), stop=False)
            nc.tensor.matmul(psu[:], ones_row[0:1, 0:1], bq_row[:],
                             start=False, stop=True)
            u_row = sp.tile([1, D], F32)
            nc.scalar.copy(u_row[:], psu[:])
            ucol = [col_of(u_row[0:1, k * P:(k + 1) * P], f"ucol{k}")
                    for k in range(2)]

            # c_qt = u @ Wk^T
            psc = spp.tile([1, D], F32, tag="row")
            for k in range(2):
                nc.tensor.matmul(psc[:], ucol[k][:], wkT[:, k, :],
                                 start=(k == 0), stop=(k == 1))
            nc.scalar.copy(cqt_row[:], psc[:])

            # wqbk = bk^T @ Wq^T (row);  a_qbk = gamma * wqbk
            psw = spp.tile([1, D], F32, tag="row")
            for k in range(2):
                nc.tensor.matmul(psw[:], bkcol[k][:], wqT[:, k, :],
                                 start=(k == 0), stop=(k == 1))
            wqbk_row = sp.tile([1, D], F32)
            nc.scalar.copy(wqbk_row[:], psw[:])
            aqbk_row = sp.tile([1, D], F32)
            nc.vector.tensor_tensor(aqbk_row[:], wqbk_row[:], gam_row[:],
                                    ALU.mult)
            for k in range(2):
                psq = spp.tile([P, 1], F32, tag="col")
                nc.tensor.transpose(psq[:], aqbk_row[0:1, k * P:(k + 1) * P],
                                    ident[0:1, 0:1])
                nc.scalar.copy(aqbkT[:, k:k + 1], psq[:])

            # c_qbk = beta.wqbk + bq.bk  -> broadcast (x 1/sqrt(D)) to [P,1]
            scr_row = sp.tile([1, D], F32)
            nc.vector.tensor_tensor(scr_row[:], bet_row[:], wqbk_row[:],
                                    ALU.mult)
            cq1 = sp.tile([1, 1], F32)
            nc.vector.tensor_reduce(cq1[:], scr_row[:], mybir.AxisListType.X,
                                    ALU.add)
            scr2_row = sp.tile([1, D], F32)
            nc.vector.tensor_tensor(scr2_row[:], bq_row[:], bkv_row[0:1, 0:D],
                                    ALU.mult)
            cq2 = sp.tile([1, 1], F32)
            nc.vector.tensor_reduce(cq2[:], scr2_row[:], mybir.AxisListType.X,
                                    ALU.add)
            cq16 = sp.tile([1, 1], F32)
            nc.vector.tensor_tensor(cq16[:], cq1[:], cq2[:], ALU.add)
            nc.vector.tensor_scalar(cq16[:], cq16[:], SCALE, None, ALU.mult)
            psb = spp.tile([P, 1], F32, tag="col")
            nc.tensor.matmul(psb[:], ones_row[0:1, 0:P], cq16[:],
                             start=True, stop=True)
            nc.scalar.copy(cqbk16_b[:], psb[:])

            # b_voWo = bv @ Wo (row)
            psv = spp.tile([1, D], F32, tag="row")
            for k in range(2):
                nc.tensor.matmul(psv[:], bvcol[k][:], wo_s[:, k, :],
                                 start=(k == 0), stop=(k == 1))
            nc.scalar.copy(bvoWo_row[:], psv[:])

        if debug:
            nc.sync.dma_start(dbg["A"][0], A_s[:, 0, :])
            nc.sync.dma_start(dbg["A"][1], A_s[:, 1, :])
            nc.sync.dma_start(dbg["cqt"][:], cqt_row[:])

        # ---------- main loop pools ----------
        big = ctx.enter_context(tc.tile_pool(name="big", bufs=2))
        mid = ctx.enter_context(tc.tile_pool(name="mid", bufs=2))
        sml = ctx.enter_context(tc.tile_pool(name="sml", bufs=4))
        ps_early = ctx.enter_context(
            tc.tile_pool(name="ps_early", bufs=1, space="PSUM"))
        ps_qt = ctx.enter_context(
            tc.tile_pool(name="ps_qt", bufs=2, space="PSUM"))
        ps_late = ctx.enter_context(
            tc.tile_pool(name="ps_late", bufs=1, space="PSUM"))
        ps_out = ctx.enter_context(
            tc.tile_pool(name="ps_out", bufs=2, space="PSUM"))

        for it in range(ntiles):
            n0 = it * P
            # ---- loads ----
            xt = sml.tile([P, D], F32, tag="xt")
            nc.sync.dma_start(xt[:], x_d[n0:n0 + P, :])
            ct = sml.tile([C, 2 * P], F32, tag="ct")
            nc.sync.dma_start(ct[:], coords_d[:, n0:n0 + P, :])
            vt_i = sml.tile([C, P], I32, tag="vti")
            nc.sync.dma_start(vt_i[:], valid_d[:, n0:n0 + P])
            vt_f = sml.tile([C, P], F32, tag="vtf")
            nc.vector.tensor_copy(vt_f[:], vt_i[:])

            # ---- transposes of coords/valid + later qbk column ----
            pse = ps_early.tile([P, 24 + 2 * P + 1], F32, tag="pse")
            pxT, pyT, vT = pse[:, 0:8], pse[:, 8:16], pse[:, 16:24]
            xcT_ps = pse[:, 24:24 + 2 * P]
            qbk_ps = pse[:, 24 + 2 * P:24 + 2 * P + 1]
            nc.tensor.transpose(pxT, ct[:, 0:2 * P:2], ident[0:C, 0:C])
            nc.tensor.transpose(pyT, ct[:, 1:2 * P:2], ident[0:C, 0:C])
            nc.tensor.transpose(vT, vt_f[:], ident[0:C, 0:C])
            valid_f = sml.tile([P, C], F32, tag="validf")
            nc.scalar.copy(valid_f[:], vT)

            # pixel coords: p = (c+1)*0.5*63 = c*31.5 + 31.5   (x cols 0:8, y 8:16)
            pxy = sml.tile([P, 2 * C], F32, tag="pxy")
            nc.scalar.activation(pxy[:], pse[:, 0:16], ACTF.Identity,
                                 bias=b315[:], scale=31.5)
            # floor(pxy) robust to cast rounding mode (trunc or nearest):
            # r = int(pxy); dx = pxy - r; neg = dx<0; floor = r-neg; frac = dx+neg
            ri = sml.tile([P, 2 * C], I32, tag="ri")
            nc.vector.tensor_copy(ri[:], pxy[:])
            rf = sml.tile([P, 2 * C], F32, tag="rf")
            nc.vector.tensor_copy(rf[:], ri[:])
            dx = sml.tile([P, 2 * C], F32, tag="dx")
            nc.vector.tensor_tensor(dx[:], pxy[:], rf[:], ALU.subtract)
            neg = sml.tile([P, 2 * C], F32, tag="neg")
            nc.vector.tensor_scalar(neg[:], dx[:], 0.0, None, ALU.is_lt)
            frac = sml.tile([P, 2 * C], F32, tag="frac")
            nc.vector.tensor_tensor(frac[:], dx[:], neg[:], ALU.add)
            xy0 = sml.tile([P, 2 * C], F32, tag="xy0")
            nc.vector.tensor_tensor(xy0[:], rf[:], neg[:], ALU.subtract)
            nc.vector.tensor_scalar(xy0[:], xy0[:], float(W - 2), None,
                                    ALU.min)   # clamp to 62 (memory safety)

            # row index r0 = cam*4096 + y0*64 + x0 ; r1 = r0 + 64
            r0f = sml.tile([P, C], F32, tag="r0f")
            nc.vector.tensor_scalar(r0f[:], xy0[:, C:2 * C], float(W), None,
                                    ALU.mult)
            nc.vector.tensor_tensor(r0f[:], r0f[:], xy0[:, 0:C], ALU.add)
            offs = sml.tile([P, 2 * C], I32, tag="offs")
            nc.vector.tensor_copy(offs[:, 0:C], r0f[:])
            nc.vector.tensor_tensor(offs[:, 0:C], offs[:, 0:C], cam_base[:],
                                    ALU.add)
            nc.vector.tensor_scalar(offs[:, C:2 * C], offs[:, 0:C], W, None,
                                    ALU.add)

            # bilinear weights: frac x = wx1, y = wy1
            wx1, wy1 = frac[:, 0:C], frac[:, C:2 * C]
            wcomp = sml.tile([P, 2, C], F32, tag="wcomp")   # [xi=0/1 compl]
            nc.vector.tensor_tensor(wcomp[:, 0, :], ones8[:], wx1, ALU.subtract)
            nc.vector.tensor_tensor(wcomp[:, 1, :], ones8[:], wy1, ALU.subtract)
            wgt = sml.tile([P, 2, 2, C], F32, tag="wgt")    # [yi, xi, c]
            nc.vector.tensor_tensor(wgt[:, 0, 0, :], wcomp[:, 1, :],
                                    wcomp[:, 0, :], ALU.mult)
            nc.vector.tensor_tensor(wgt[:, 0, 1, :], wcomp[:, 1, :], wx1,
                                    ALU.mult)
            nc.vector.tensor_tensor(wgt[:, 1, 0, :], wy1, wcomp[:, 0, :],
                                    ALU.mult)
            nc.vector.tensor_tensor(wgt[:, 1, 1, :], wy1, wx1, ALU.mult)

            # ---- the gather: 16 indirect DMAs (one per camera x y-row);
            # HW consumes exactly one offset per dest partition, streaming
            # 512 contiguous floats (the x-pair) from each row base.
            tgh = [big.tile([P, C, 2, D], F32, tag="tg0", name="tg0"),
                   big.tile([P, C, 2, D], F32, tag="tg1", name="tg1")]
            for yi in range(2):
                for c in range(C):
                    j = yi * C + c
                    nc.gpsimd.indirect_dma_start(
                        out=tgh[yi][:, c, :, :].rearrange("p a d -> p (a d)"),
                        out_offset=None, in_=feat_d[:],
                        in_offset=bass.IndirectOffsetOnAxis(
                            ap=offs[:, j:j + 1], axis=0))

            # ---- layernorm + qt ----
            stats = sml.tile([P, 6], F32, tag="stats")
            nc.vector.bn_stats(stats[:], xt[:])
            mv = sml.tile([P, 2], F32, tag="mv")
            nc.vector.bn_aggr(mv[:], stats[:])
            rs = sml.tile([P, 1], F32, tag="rs")
            nc.scalar.activation(rs[:], mv[:, 1:2], ACTF.Sqrt, bias=epsb[:],
                                 scale=1.0)
            nc.vector.reciprocal(rs[:], rs[:])
            # z = (x - mu) * rs  (full LN normalization before the matmul, so
            # the c_qt column constant folded into PSUM is NOT rs-scaled)
            xc = sml.tile([P, D], F32, tag="xc")
            nc.vector.tensor_scalar(xc[:], xt[:], mv[:, 0:1], rs[:],
                                    ALU.subtract, ALU.mult)
            for j in range(2):
                nc.tensor.transpose(xcT_ps[:, j * P:(j + 1) * P],
                                    xc[:, j * P:(j + 1) * P], ident[:])
            xcT = sml.tile([P, 2 * P], F32, tag="xcT")
            nc.scalar.copy(xcT[:], xcT_ps)

            qt_ps = ps_qt.tile([P, D], F32, tag="qt")
            for j in range(2):
                nc.tensor.matmul(qt_ps[:], xcT[:, j * P:(j + 1) * P],
                                 A_s[:, j, :], start=(j == 0), stop=False)
            nc.tensor.matmul(qt_ps[:], ones_row[0:1, 0:P], cqt_row[:],
                             start=False, stop=True)
            for j in range(2):
                nc.tensor.matmul(qbk_ps, xcT[:, j * P:(j + 1) * P],
                                 aqbkT[:, j:j + 1], start=(j == 0),
                                 stop=(j == 1))
            qt_sb = sml.tile([P, D], DT, tag="qtsb")
            nc.scalar.copy(qt_sb[:], qt_ps[:])
            qbk_sb = sml.tile([P, 1], F32, tag="qbksb")
            nc.scalar.copy(qbk_sb[:], qbk_ps)
            nc.vector.tensor_scalar(qbk_sb[:], qbk_sb[:], SCALE, None,
                                    ALU.mult)
            nc.vector.tensor_tensor(qbk_sb[:], qbk_sb[:], cqbk16_b[:], ALU.add)
            # per-element bias so the ACT accumulate adds qbk/sqrt(D) overall
            nc.vector.tensor_scalar(qbk_sb[:], qbk_sb[:], 1.0 / D, None,
                                    ALU.mult)

            # ---- scale gathered corners by bilinear weights -> DT ----
            # bf16: write converted copy; fp32: scale in place (SBUF budget)
            tb = mid.tile([P, 2, C, 2, D], DT, tag="tb", name="tb")
            k = 0
            for yi in range(2):
                for c in range(C):
                    for xi in range(2):
                        eng = nc.vector if (k % 8) < 5 else nc.scalar
                        wap = wgt[:, yi, xi, c:c + 1]
                        src = tgh[yi][:, c, xi, :]
                        dst = tb[:, yi, c, xi, :]
                        if eng is nc.scalar:
                            nc.scalar.activation(dst, src, ACTF.Copy,
                                                 bias=0.0, scale=wap)
                        else:
                            eng.tensor_scalar(dst, src, wap, None, ALU.mult)
                        k += 1

            # ---- corner reduce: s = sum over (yi, xi) ----
            u = mid.tile([P, C, 2, D], DT, tag="u")
            nc.vector.tensor_tensor(u[:], tb[:, 0], tb[:, 1], ALU.add)
            s = mid.tile([P, C, D], DT, tag="s")
            nc.vector.tensor_tensor(s[:], u[:, :, 0, :], u[:, :, 1, :],
                                    ALU.add)

            # ---- scores: qt.s dots via DVE mult + ACT accumulate ----
            scores = sml.tile([P, C], F32, tag="scores")
            prod = mid.tile([P, C, D], DT, tag="prod")
            qt_b = qt_sb.rearrange("p (a d) -> p a d", a=1).to_broadcast(
                [P, C, D])
            nc.vector.tensor_tensor(prod[:], s[:], qt_b, ALU.mult)
            ttr_scr = sml.tile([P, D], F32, tag="ttrscr")
            for c in range(C):
                nc.scalar.activation(ttr_scr[:], prod[:, c, :], ACTF.Identity,
                                     bias=qbk_sb[:], scale=SCALE,
                                     accum_out=scores[:, c:c + 1])

            # ---- masked softmax over cameras ----
            vneg = sml.tile([P, C], F32, tag="vneg")
            nc.vector.tensor_scalar(vneg[:], valid_f[:], 1e30, 1e30,
                                    ALU.mult, ALU.subtract)
            masked = sml.tile([P, C], F32, tag="masked")
            nc.vector.tensor_tensor(masked[:], scores[:], valid_f[:], ALU.mult)
            nc.vector.tensor_tensor(masked[:], masked[:], vneg[:], ALU.add)
            negm = sml.tile([P, 1], F32, tag="negm")
            nc.vector.tensor_reduce(negm[:], masked[:], mybir.AxisListType.X,
                                    ALU.max, negate=True)
            expd = sml.tile([P, C], F32, tag="expd")
            nc.scalar.activation(expd[:], masked[:], ACTF.Exp, bias=negm[:],
                                 scale=1.0)
            nc.vector.tensor_tensor(expd[:], expd[:], valid_f[:], ALU.mult)
            ssum = sml.tile([P, 1], F32, tag="ssum")
            nc.vector.tensor_reduce(ssum[:], expd[:], mybir.AxisListType.X,
                                    ALU.add)
            nc.vector.tensor_scalar(ssum[:], ssum[:], 1e-30, None, ALU.add)
            nc.vector.reciprocal(ssum[:], ssum[:])
            attn = sml.tile([P, C], F32, tag="attn")
            nc.vector.tensor_scalar(attn[:], expd[:], ssum[:], None, ALU.mult)
            sum_attn = sml.tile([P, 1], F32, tag="sumattn")
            nc.vector.tensor_reduce(sum_attn[:], attn[:], mybir.AxisListType.X,
                                    ALU.add)

            # ---- g = sum_c attn_c * s_c ----
            sw = mid.tile([P, C, D], DT, tag="sw")
            attn_b = attn.rearrange("p (c a) -> p c a", a=1).to_broadcast(
                [P, C, D])
            nc.vector.tensor_tensor(sw[:], s[:], attn_b, ALU.mult)
            g4 = mid.tile([P, 4, D], DT, tag="g4")
            nc.vector.tensor_tensor(g4[:], sw[:, 0:4, :], sw[:, 4:8, :],
                                    ALU.add)
            g2 = sml.tile([P, 2, D], DT, tag="g2")
            nc.vector.tensor_tensor(g2[:], g4[:, 0:2, :], g4[:, 2:4, :],
                                    ALU.add)
            g = sml.tile([P, D], F32, tag="g")
            nc.vector.tensor_tensor(g[:], g2[:, 0, :], g2[:, 1, :], ALU.add)

            # ---- final: out = x + g@Wvo + sum_attn*bvoWo + bo ----
            psl = ps_late.tile([P, 2 * P + P], F32, tag="psl")
            gT_ps = psl[:, 0:2 * P]
            saT_ps = psl[0:1, 2 * P:3 * P]
            for j in range(2):
                nc.tensor.transpose(gT_ps[:, j * P:(j + 1) * P],
                                    g[:, j * P:(j + 1) * P], ident[:])
            nc.tensor.transpose(saT_ps, sum_attn[:], ident[:])
            gT = sml.tile([P, 2 * P], F32, tag="gT")
            nc.scalar.copy(gT[:], gT_ps)
            saT = sml.tile([1, P], F32, tag="saT")
            nc.scalar.copy(saT[:], saT_ps)

            out_ps = ps_out.tile([P, D], F32, tag="out")
            for j in range(2):
                nc.tensor.matmul(out_ps[:], gT[:, j * P:(j + 1) * P],
                                 Wvo_s[:, j, :], start=(j == 0), stop=False)
            nc.tensor.matmul(out_ps[:], saT[:], bvoWo_row[:], start=False,
                             stop=False)
            nc.tensor.matmul(out_ps[:], ones_row[0:1, 0:P], bo_row[:],
                             start=False, stop=True)
            if debug:
                nc.sync.dma_start(dbg["offs"][n0:n0 + P, :], offs[:])
                nc.sync.dma_start(dbg["wgt"][n0:n0 + P, :],
                                  wgt.rearrange("p a b c -> p (a b c)"))
                if it == 0:
                    nc.sync.dma_start(dbg["tg0"][:], tg[:, :, 0, :, :])
                    nc.sync.dma_start(dbg["camb"][:], cam_base[:])
                nc.sync.dma_start(dbg["xcT"][it], xcT[:])
                qtps_f = sml.tile([P, D], F32, tag="dbgqtps")
                nc.vector.tensor_copy(qtps_f[:], qt_ps[:])
                nc.sync.dma_start(dbg["qtps"][n0:n0 + P, :], qtps_f[:])
                nc.sync.dma_start(dbg["mv"][n0:n0 + P, :], mv[:])
                nc.sync.dma_start(dbg["rs"][n0:n0 + P, :], rs[:])
                qt_f = sml.tile([P, D], F32, tag="dbgqt")
                nc.vector.tensor_copy(qt_f[:], qt_sb[:])
                nc.sync.dma_start(dbg["qt"][n0:n0 + P, :], qt_f[:])
                nc.sync.dma_start(dbg["scores"][n0:n0 + P, :], scores[:])
                nc.sync.dma_start(dbg["attn"][n0:n0 + P, :], attn[:])
                s0f = sml.tile([P, D], F32, tag="dbgs0")
                nc.vector.tensor_copy(s0f[:], s[:, 0, :])
                nc.sync.dma_start(dbg["s0"][n0:n0 + P, :], s0f[:])
                nc.sync.dma_start(dbg["g"][n0:n0 + P, :], g[:])
            out_sb = sml.tile([P, D], F32, tag="outsb")
            nc.vector.tensor_tensor(out_sb[:], out_ps[:], xt[:], ALU.add)
            nc.sync.dma_start(out_d[n0:n0 + P, :], out_sb[:])

    return nc


# ---------------------------------------------------------------------------
# Post-scheduling legalization: the walrus build here encodes at most ONE
# sync-wait command per TPB instruction (matmul LDWEIGHTS / CTRL structs
# reject more). Hoist excess waits onto same-engine EventSemaphore helpers
# inserted immediately before the offending instruction (sequencer order
# preserves blocking semantics exactly).
_LGL_UID = [0]


def legalize_waits(nc, cap=1):
    n_helpers = 0
    for fn in nc.m.functions:
        for bb in fn.blocks:
            out = []
            for ins in bb.instructions:
                si = ins.sync_info
                waits = list(si.on_wait) if si is not None else []
                if len(waits) > cap:
                    excess, keep = waits[:-cap], waits[-cap:]
                    for w in excess:
                        _LGL_UID[0] += 1
                        helper = mybir.InstEventSemaphore(
                            name=f"I-lgl-{_LGL_UID[0]}", ins=[], outs=[])
                        helper.engine = ins.engine
                        helper.sync_info = mybir.SyncInfo(
                            on_wait=[w], on_update=[])
                        out.append(helper)
                        n_helpers += 1
                    ins.sync_info = mybir.SyncInfo(
                        on_wait=keep,
                        on_update=list(si.on_update) if si is not None else [])
                out.append(ins)
            bb.instructions = out
    return n_helpers



_NC_CACHE = {}


def _get_nc(rows_per_core=ROWS_PER_CORE, use_bf16=USE_BF16):
    key = (rows_per_core, use_bf16)
    if key not in _NC_CACHE:
        nc = build_nc(rows_per_core, use_bf16)
        legalize_waits(nc)
        _NC_CACHE[key] = nc
    return _NC_CACHE[key]


def make_in_maps(inputs, rows_per_core=ROWS_PER_CORE, ncores=NCORES):
    q = np.ascontiguousarray(np.asarray(inputs["queries"], np.float32))
    feat = np.ascontiguousarray(
        np.asarray(inputs["image_features"], np.float32))
    pc = np.ascontiguousarray(np.asarray(inputs["pixel_coords"], np.float32))
    vm = np.ascontiguousarray(np.asarray(inputs["valid_mask"], np.int32))
    wshared = {
        "Wq": np.ascontiguousarray(np.asarray(inputs["Wq"], np.float32)),
        "bq": np.ascontiguousarray(np.asarray(inputs["bq"], np.float32)),
        "Wkv": np.ascontiguousarray(np.asarray(inputs["Wkv"], np.float32)),
        "bkv": np.ascontiguousarray(np.asarray(inputs["bkv"], np.float32)),
        "Wo": np.ascontiguousarray(np.asarray(inputs["Wo"], np.float32)),
        "bo": np.ascontiguousarray(np.asarray(inputs["bo"], np.float32)),
        "gamma": np.ascontiguousarray(np.asarray(inputs["gamma"], np.float32)),
        "beta": np.ascontiguousarray(np.asarray(inputs["beta"], np.float32)),
    }
    in_maps = []
    feat_flat = [np.ascontiguousarray(feat[b].reshape(C * H * W, D))
                 for b in range(B)]
    per_b = N // (ncores // B)   # rows handled per core within a batch
    for core in range(ncores):
        b = core // (ncores // B)
        n0 = (core % (ncores // B)) * per_b
        m = {
            "x": np.ascontiguousarray(q[b, n0:n0 + rows_per_core, :]),
            "feat": feat_flat[b],
            "coords": np.ascontiguousarray(pc[b, :, n0:n0 + rows_per_core, :]),
            "valid": np.ascontiguousarray(vm[b, :, n0:n0 + rows_per_core]),
        }
        m.update(wshared)
        in_maps.append(m)
    return in_maps


def kernel(**inputs) -> np.ndarray:
    from concourse.bass_utils import run_bass_kernel_spmd
    nc = _get_nc()
    in_maps = make_in_maps(inputs)
    res = run_bass_kernel_spmd(nc, in_maps, core_ids=list(range(NCORES)))
    outs = [np.asarray(r["out"]) for r in res.results]
    full = np.concatenate(outs, axis=0).reshape(B, N, D)
    return full.astype(np.float32)



# revision 3
# speedup vs baseline: 2.3426x; 2.3426x over previous
"""Trainium2 Bass kernel for nn_CrossImageAttentionLayer (v3).

Contract: kernel(**inputs) takes FULL unsharded inputs (as produced by
setup_inputs) and returns the FULL (B, N, D) output. Internally shards the
flattened (B*N) query dimension across 8 NeuronCores (all-reduce-free),
builds one SPMD Bass/Tile kernel, and runs it via run_bass_kernel_spmd.

v3 structure:
  - image features are repacked on the host into overlapping 2x2 pixel
    blocks in bf16: blk[c*4096 + y*64 + x] = feat[c, y:y+2, x:x+2] flat
    (a=yi, b=xi, d).  One bilinear footprint == one contiguous 2KB read,
    so each (query, camera) pair costs ONE descriptor.
  - valid-mask compaction: queries are sorted per core by their number of
    valid cameras and each query's valid cameras are compacted into the
    leading "slots".  Each 128-query tile then only gathers / computes
    max-valid-in-tile slots (~4.7 avg instead of 8): ~40% fewer HBM
    bytes, SWDGE descriptor-generation calls, and vector-engine
    elements.  The slot schedule is derived from the actual input and
    compiled in (nc cache keyed by schedule); outputs are inverse-
    permuted on the host.
  - bilinear corner offsets (int32) and weights (bf16, zeroed on invalid
    slots) are precomputed host-side; valid mask pre-cast to f32.
  - scores via ACT-accumulate (frees DVE); qt/out matmuls in bf16.

Per-tile pipeline (128 queries, K slots):
  layernorm -> xc -> (PE bf16) qt = xc@A + c_qt, qbk = xc@aqbk
  K x indirect DMA -> tg[P, j, (a,b,d)] bf16
  tg *= w4 (4 broadcast multiplies, in place); u = a0+a1; s = b0+b1
  prod = s * qt ; scores[:, j] = ACT-accum(SCALE*prod + qbk/D)
  masked softmax over K slots; g = sum_j attn_j * s_j (pairwise tree)
  out = x + g@Wvo + sum_attn*(bv@Wo) + bo
"""
import math
import numpy as np
from contextlib import ExitStack

import concourse.bass as bass
import concourse.tile as tile
import concourse.mybir as mybir
from concourse.bass import AP
from concourse.masks import make_identity

F32 = mybir.dt.float32
BF16 = mybir.dt.bfloat16
I32 = mybir.dt.int32
ALU = mybir.AluOpType
ACTF = mybir.ActivationFunctionType

# problem constants (hardcoded per harness contract)
B, C, N, D, H, W = 2, 8, 16384, 256, 64, 64
NCORES = 8
ROWS_PER_CORE = (B * N) // NCORES      # 4096
P = 128                                # tile = 128 queries
NTILES = ROWS_PER_CORE // P
EPS = 1e-5
SCALE = 1.0 / math.sqrt(float(D))
BLK = 4 * D                            # 2x2 block = 1024 elements


def build_nc(schedule, rows_per_core=ROWS_PER_CORE):
    """schedule: per-tile slot counts (len == ntiles), each in 1..C."""
    ntiles = rows_per_core // P
    assert len(schedule) == ntiles
    nc = bass.Bass()

    xv_d = nc.dram_tensor("xv", (rows_per_core, D + C), F32,
                          kind="ExternalInput")          # queries | valid
    blk_d = nc.dram_tensor("blk", (C * H * W, BLK), BF16,
                           kind="ExternalInput")         # 2x2 pixel blocks
    offs_d = nc.dram_tensor("offs", (rows_per_core, C), I32,
                            kind="ExternalInput")        # block row index
    w4_d = nc.dram_tensor("w4", (rows_per_core, 4 * C), BF16,
                          kind="ExternalInput")          # (ab, slot) weights
    wq_d = nc.dram_tensor("Wq", (D, D), F32, kind="ExternalInput")
    bq_d = nc.dram_tensor("bq", (D,), F32, kind="ExternalInput")
    wkv_d = nc.dram_tensor("Wkv", (D, 2 * D), F32, kind="ExternalInput")
    bkv_d = nc.dram_tensor("bkv", (2 * D,), F32, kind="ExternalInput")
    wo_d = nc.dram_tensor("Wo", (D, D), F32, kind="ExternalInput")
    bo_d = nc.dram_tensor("bo", (D,), F32, kind="ExternalInput")
    gamma_d = nc.dram_tensor("gamma", (D,), F32, kind="ExternalInput")
    beta_d = nc.dram_tensor("beta", (D,), F32, kind="ExternalInput")
    out_d = nc.dram_tensor("out", (rows_per_core, D), F32,
                           kind="ExternalOutput")

    with tile.TileContext(nc) as tc, ExitStack() as ctx:
        const = ctx.enter_context(tc.tile_pool(name="const", bufs=1))

        # ---------- constants ----------
        ident = const.tile([P, P], F32)
        make_identity(nc, ident[:])
        ones_row = const.tile([1, D], F32)
        nc.vector.memset(ones_row[:], 1.0)
        epsb = const.tile([P, 1], F32)
        nc.vector.memset(epsb[:], EPS)

        # ---------- persistent weight products ----------
        A_s = const.tile([P, 2, D], BF16)      # diag(gamma) Wq Wk^T (bf16)
        Wvo_s = const.tile([P, 2, D], BF16)    # Wv @ Wo (bf16)
        cqt_row = const.tile([1, D], F32)
        aqbkT = const.tile([P, 2], F32)        # (gamma*Wq@bk) as column slabs
        bvoWo_row = const.tile([1, D], F32)
        bo_row = const.tile([1, D], F32)
        cqbk16_b = const.tile([P, 1], F32)     # (beta@Wq@bk + bq.bk)/sqrt(D) bcast

        with tc.tile_pool(name="setup", bufs=1) as sp, \
             tc.tile_pool(name="setup_ps", bufs=1, space="PSUM") as spp:
            wq_s = sp.tile([P, 2, D], F32)
            nc.sync.dma_start(wq_s[:], wq_d.rearrange("(j p) d -> p j d", p=P))
            wkv_s = sp.tile([P, 2, 2 * D], F32)
            nc.sync.dma_start(wkv_s[:], wkv_d.rearrange("(j p) d -> p j d", p=P))
            wo_s = sp.tile([P, 2, D], F32)
            nc.sync.dma_start(wo_s[:], wo_d.rearrange("(j p) d -> p j d", p=P))
            nc.sync.dma_start(bo_row[:], bo_d[None, :])
            gam_row = sp.tile([1, D], F32)
            nc.sync.dma_start(gam_row[:], gamma_d[None, :])
            bet_row = sp.tile([1, D], F32)
            nc.sync.dma_start(bet_row[:], beta_d[None, :])
            bq_row = sp.tile([1, D], F32)
            nc.sync.dma_start(bq_row[:], bq_d[None, :])
            bkv_row = sp.tile([1, 2 * D], F32)
            nc.sync.dma_start(bkv_row[:], bkv_d[None, :])

            def transpose_256(dst, src):
                for jj in range(2):
                    pst = spp.tile([P, P], F32, tag="tp")
                    for i in range(2):
                        nc.tensor.transpose(
                            pst[:], src[:, i, jj * P:(jj + 1) * P], ident[:])
                        nc.scalar.copy(dst[:, jj, i * P:(i + 1) * P], pst[:])
                        if i == 0:
                            pst = spp.tile([P, P], F32, tag="tp")

            wqT = sp.tile([P, 2, D], F32)
            transpose_256(wqT, wq_s)
            wkT = sp.tile([P, 2, D], F32)
            transpose_256(wkT, wkv_s[:, :, 0:D])
            wvT = sp.tile([P, 2, D], F32)
            transpose_256(wvT, wkv_s[:, :, D:2 * D])

            def col_of(row_ap, tag):
                ps = spp.tile([P, 1], F32, tag="col")
                nc.tensor.transpose(ps[:], row_ap, ident[0:1, 0:1])
                sb = sp.tile([P, 1], F32, tag=tag)
                nc.scalar.copy(sb[:], ps[:])
                return sb

            gcol = [col_of(gam_row[0:1, k * P:(k + 1) * P], f"gcol{k}")
                    for k in range(2)]
            betcol = [col_of(bet_row[0:1, k * P:(k + 1) * P], f"betcol{k}")
                      for k in range(2)]
            bkcol = [col_of(bkv_row[0:1, k * P:(k + 1) * P], f"bkcol{k}")
                     for k in range(2)]
            bvcol = [col_of(bkv_row[0:1, D + k * P:D + (k + 1) * P],
                            f"bvcol{k}") for k in range(2)]

            # A = Wq @ Wk^T (M-blocks i), then scale rows by gamma into A_s
            for i in range(2):
                psA = spp.tile([P, D], F32, tag="mm")
                for k in range(2):
                    nc.tensor.matmul(psA[:], wqT[:, k, i * P:(i + 1) * P],
                                     wkT[:, k, :], start=(k == 0),
                                     stop=(k == 1))
                nc.vector.tensor_scalar(A_s[:, i, :], psA[:], gcol[i][:],
                                        None, ALU.mult)

            # W_vo = Wv @ Wo
            for i in range(2):
                psV = spp.tile([P, D], F32, tag="mm")
                for k in range(2):
                    nc.tensor.matmul(psV[:], wvT[:, k, i * P:(i + 1) * P],
                                     wo_s[:, k, :], start=(k == 0),
                                     stop=(k == 1))
                nc.scalar.copy(Wvo_s[:, i, :], psV[:])

            # u = beta@Wq + bq  (row)
            psu = spp.tile([1, D], F32, tag="row")
            for k in range(2):
                nc.tensor.matmul(psu[:], betcol[k][:], wq_s[:, k, :],
                                 start=(k == 0), stop=False)
            nc.tensor.matmul(psu[:], ones_row[0:1, 0:1], bq_row[:],
                             start=False, stop=True)
            u_row = sp.tile([1, D], F32)
            nc.scalar.copy(u_row[:], psu[:])
            ucol = [col_of(u_row[0:1, k * P:(k + 1) * P], f"ucol{k}")
                    for k in range(2)]

            # c_qt = u @ Wk^T
            psc = spp.tile([1, D], F32, tag="row")
            for k in range(2):
                nc.tensor.matmul(psc[:], ucol[k][:], wkT[:, k, :],
                                 start=(k == 0), stop=(k == 1))
            nc.scalar.copy(cqt_row[:], psc[:])

            # wqbk = bk^T @ Wq^T (row);  a_qbk = gamma * wqbk
            psw = spp.tile([1, D], F32, tag="row")
            for k in range(2):
                nc.tensor.matmul(psw[:], bkcol[k][:], wqT[:, k, :],
                                 start=(k == 0), stop=(k == 1))
            wqbk_row = sp.tile([1, D], F32)
            nc.scalar.copy(wqbk_row[:], psw[:])
            aqbk_row = sp.tile([1, D], F32)
            nc.vector.tensor_tensor(aqbk_row[:], wqbk_row[:], gam_row[:],
                                    ALU.mult)
            for k in range(2):
                psq = spp.tile([P, 1], F32, tag="col")
                nc.tensor.transpose(psq[:], aqbk_row[0:1, k * P:(k + 1) * P],
                                    ident[0:1, 0:1])
                nc.scalar.copy(aqbkT[:, k:k + 1], psq[:])

            # c_qbk = beta.wqbk + bq.bk  -> broadcast (x 1/sqrt(D)) to [P,1]
            scr_row = sp.tile([1, D], F32)
            nc.vector.tensor_tensor(scr_row[:], bet_row[:], wqbk_row[:],
                                    ALU.mult)
            cq1 = sp.tile([1, 1], F32)
            nc.vector.tensor_reduce(cq1[:], scr_row[:], mybir.AxisListType.X,
                                    ALU.add)
            scr2_row = sp.tile([1, D], F32)
            nc.vector.tensor_tensor(scr2_row[:], bq_row[:], bkv_row[0:1, 0:D],
                                    ALU.mult)
            cq2 = sp.tile([1, 1], F32)
            nc.vector.tensor_reduce(cq2[:], scr2_row[:], mybir.AxisListType.X,
                                    ALU.add)
            cq16 = sp.tile([1, 1], F32)
            nc.vector.tensor_tensor(cq16[:], cq1[:], cq2[:], ALU.add)
            nc.vector.tensor_scalar(cq16[:], cq16[:], SCALE, None, ALU.mult)
            psb = spp.tile([P, 1], F32, tag="col")
            nc.tensor.matmul(psb[:], ones_row[0:1, 0:P], cq16[:],
                             start=True, stop=True)
            nc.scalar.copy(cqbk16_b[:], psb[:])

            # b_voWo = bv @ Wo (row)
            psv = spp.tile([1, D], F32, tag="row")
            for k in range(2):
                nc.tensor.matmul(psv[:], bvcol[k][:], wo_s[:, k, :],
                                 start=(k == 0), stop=(k == 1))
            nc.scalar.copy(bvoWo_row[:], psv[:])

        # bf16 copies of row constants for the bf16 matmul chains
        cqt_row16 = const.tile([1, D], BF16)
        nc.vector.tensor_copy(cqt_row16[:], cqt_row[:])
        bvoWo_row16 = const.tile([1, D], BF16)
        nc.vector.tensor_copy(bvoWo_row16[:], bvoWo_row[:])
        bo_row16 = const.tile([1, D], BF16)
        nc.vector.tensor_copy(bo_row16[:], bo_row[:])
        ones_row16 = const.tile([1, D], BF16)
        nc.vector.tensor_copy(ones_row16[:], ones_row[:])
        ident16 = const.tile([P, P], BF16)
        nc.vector.tensor_copy(ident16[:], ident[:])
        aqbkT16 = const.tile([P, 2], BF16)
        nc.vector.tensor_copy(aqbkT16[:], aqbkT[:])

        # ---------- main loop pools ----------
        big = ctx.enter_context(tc.tile_pool(name="big", bufs=2))
        mid = ctx.enter_context(tc.tile_pool(name="mid", bufs=2))
        sml = ctx.enter_context(tc.tile_pool(name="sml", bufs=4))
        ps_early = ctx.enter_context(
            tc.tile_pool(name="ps_early", bufs=2, space="PSUM"))
        ps_qt = ctx.enter_context(
            tc.tile_pool(name="ps_qt", bufs=2, space="PSUM"))
        ps_late = ctx.enter_context(
            tc.tile_pool(name="ps_late", bufs=2, space="PSUM"))
        ps_out = ctx.enter_context(
            tc.tile_pool(name="ps_out", bufs=2, space="PSUM"))

        for it in range(ntiles):
            n0 = it * P
            K = int(schedule[it])
            # ---- loads ----
            xvt = sml.tile([P, D + C], F32, tag="xvt")
            nc.sync.dma_start(xvt[:], xv_d[n0:n0 + P, :])
            xt = xvt[:, 0:D]
            valid_f = xvt[:, D:D + K]
            offt = sml.tile([P, C], I32, tag="offt")
            nc.sync.dma_start(offt[:], offs_d[n0:n0 + P, :])
            w4t = sml.tile([P, 4 * C], BF16, tag="w4t")
            nc.sync.dma_start(w4t[:], w4_d[n0:n0 + P, :])

            # ---- the gather: one 2KB block per (query, slot) ----
            tg = big.tile([P, C, BLK], BF16, tag="tg", name="tg")
            for j in range(K):
                nc.gpsimd.indirect_dma_start(
                    out=tg[:, j, :],
                    out_offset=None, in_=blk_d[:],
                    in_offset=bass.IndirectOffsetOnAxis(
                        ap=offt[:, j:j + 1], axis=0))

            # ---- layernorm + qt (bf16 PE) ----
            stats = sml.tile([P, 6], F32, tag="stats")
            nc.vector.bn_stats(stats[:], xt)
            mv = sml.tile([P, 2], F32, tag="mv")
            nc.vector.bn_aggr(mv[:], stats[:])
            rs = sml.tile([P, 1], F32, tag="rs")
            nc.scalar.activation(rs[:], mv[:, 1:2], ACTF.Sqrt, bias=epsb[:],
                                 scale=1.0)
            nc.vector.reciprocal(rs[:], rs[:])
            xc = sml.tile([P, D], F32, tag="xc")
            nc.vector.tensor_scalar(xc[:], xt, mv[:, 0:1], rs[:],
                                    ALU.subtract, ALU.mult)
            pse = ps_early.tile([P, 2 * P + 1], F32, tag="pse")
            xcT_ps = pse[:, 0:2 * P]
            qbk_ps = pse[:, 2 * P:2 * P + 1]
            for j in range(2):
                nc.tensor.transpose(xcT_ps[:, j * P:(j + 1) * P],
                                    xc[:, j * P:(j + 1) * P], ident[:])
            xcT = sml.tile([P, 2 * P], BF16, tag="xcT")
            nc.scalar.copy(xcT[:], xcT_ps)

            qt_ps = ps_qt.tile([P, D], F32, tag="qt")
            for j in range(2):
                nc.tensor.matmul(qt_ps[:], xcT[:, j * P:(j + 1) * P],
                                 A_s[:, j, :], start=(j == 0), stop=False)
            nc.tensor.matmul(qt_ps[:], ones_row16[0:1, 0:P], cqt_row16[:],
                             start=False, stop=True)
            for j in range(2):
                nc.tensor.matmul(qbk_ps, xcT[:, j * P:(j + 1) * P],
                                 aqbkT16[:, j:j + 1], start=(j == 0),
                                 stop=(j == 1))
            qt_sb = sml.tile([P, D], BF16, tag="qtsb")
            nc.scalar.copy(qt_sb[:], qt_ps[:])
            # qbk bias per element: (qbk*SCALE + cqbk16)/D
            qbk_col = sml.tile([P, 1], F32, tag="qbkcol")
            nc.scalar.copy(qbk_col[:], qbk_ps)
            nc.vector.tensor_scalar(qbk_col[:], qbk_col[:], SCALE,
                                    cqbk16_b[:], ALU.mult, ALU.add)
            nc.vector.tensor_scalar(qbk_col[:], qbk_col[:], 1.0 / D, None,
                                    ALU.mult)

            # ---- bilinear: weight the 4 corners (in place), then reduce ----
            for ab in range(4):
                wb = w4t[:, ab * C:ab * C + K].rearrange(
                    "p (c a) -> p c a", a=1).to_broadcast([P, K, D])
                seg = tg[:, 0:K, ab * D:(ab + 1) * D]
                nc.vector.tensor_tensor(seg, seg, wb, ALU.mult)
            nc.vector.tensor_tensor(tg[:, 0:K, 0:2 * D], tg[:, 0:K, 0:2 * D],
                                    tg[:, 0:K, 2 * D:4 * D], ALU.add)
            s = mid.tile([P, C, D], BF16, tag="s")
            nc.vector.tensor_tensor(s[:, 0:K, :], tg[:, 0:K, 0:D],
                                    tg[:, 0:K, D:2 * D], ALU.add)

            # ---- scores: ACT accumulate of SCALE*prod + qbk/D per slot ----
            prod = mid.tile([P, C, D], BF16, tag="prod")
            qt_b = qt_sb.rearrange("p (a d) -> p a d", a=1).to_broadcast(
                [P, K, D])
            nc.vector.tensor_tensor(prod[:, 0:K, :], s[:, 0:K, :], qt_b,
                                    ALU.mult)
            scores = sml.tile([P, C], F32, tag="scores")
            scr = sml.tile([P, D], F32, tag="actscr")
            for j in range(K):
                nc.scalar.activation(scr[:], prod[:, j, :], ACTF.Identity,
                                     bias=qbk_col[:], scale=SCALE,
                                     accum_out=scores[:, j:j + 1])

            # ---- masked softmax over K slots ----
            vneg = sml.tile([P, C], F32, tag="vneg")
            nc.vector.tensor_scalar(vneg[:, 0:K], valid_f, 1e30, 1e30,
                                    ALU.mult, ALU.subtract)
            masked = sml.tile([P, C], F32, tag="masked")
            nc.vector.tensor_tensor(masked[:, 0:K], scores[:, 0:K], valid_f,
                                    ALU.mult)
            nc.vector.tensor_tensor(masked[:, 0:K], masked[:, 0:K],
                                    vneg[:, 0:K], ALU.add)
            negm = sml.tile([P, 1], F32, tag="negm")
            nc.vector.tensor_reduce(negm[:], masked[:, 0:K],
                                    mybir.AxisListType.X, ALU.max, negate=True)
            expd = sml.tile([P, C], F32, tag="expd")
            nc.scalar.activation(expd[:, 0:K], masked[:, 0:K], ACTF.Exp,
                                 bias=negm[:], scale=1.0)
            nc.vector.tensor_tensor(expd[:, 0:K], expd[:, 0:K], valid_f,
                                    ALU.mult)
            ssum = sml.tile([P, 1], F32, tag="ssum")
            nc.vector.tensor_reduce(ssum[:], expd[:, 0:K],
                                    mybir.AxisListType.X, ALU.add)
            nc.vector.tensor_scalar(ssum[:], ssum[:], 1e-30, None, ALU.add)
            nc.vector.reciprocal(ssum[:], ssum[:])
            attn = sml.tile([P, C], F32, tag="attn")
            nc.vector.tensor_scalar(attn[:, 0:K], expd[:, 0:K], ssum[:], None,
                                    ALU.mult)
            sum_attn = sml.tile([P, 1], F32, tag="sumattn")
            nc.vector.tensor_reduce(sum_attn[:], attn[:, 0:K],
                                    mybir.AxisListType.X, ALU.add)

            # ---- g = sum_j attn_j * s_j (pairwise tree over K slots) ----
            sw = mid.tile([P, C, D], BF16, tag="sw")
            attn_b = attn.rearrange("p (c a) -> p c a", a=1)[:, 0:K, :] \
                .to_broadcast([P, K, D])
            nc.vector.tensor_tensor(sw[:, 0:K, :], s[:, 0:K, :], attn_b,
                                    ALU.mult)
            k = K
            while k > 2:
                h = (k + 1) // 2
                nc.vector.tensor_tensor(sw[:, 0:k - h, :], sw[:, 0:k - h, :],
                                        sw[:, h:k, :], ALU.add)
                k = h
            g = sml.tile([P, D], F32, tag="g")
            if k == 2:
                nc.vector.tensor_tensor(g[:], sw[:, 0, :], sw[:, 1, :],
                                        ALU.add)
            else:
                nc.vector.tensor_copy(g[:], sw[:, 0, :])

            # ---- final: out = x + g@Wvo + sum_attn*bvoWo + bo (bf16 PE) ----
            psl = ps_late.tile([P, 2 * P + P], F32, tag="psl")
            gT_ps = psl[:, 0:2 * P]
            saT_ps = psl[0:1, 2 * P:3 * P]
            for j in range(2):
                nc.tensor.transpose(gT_ps[:, j * P:(j + 1) * P],
                                    g[:, j * P:(j + 1) * P], ident[:])
            nc.tensor.transpose(saT_ps, sum_attn[:], ident[:])
            gT = sml.tile([P, 2 * P], BF16, tag="gT")
            nc.scalar.copy(gT[:], gT_ps)
            saT = sml.tile([1, P], BF16, tag="saT")
            nc.scalar.copy(saT[:], saT_ps)

            out_ps = ps_out.tile([P, D], F32, tag="out")
            for j in range(2):
                nc.tensor.matmul(out_ps[:], gT[:, j * P:(j + 1) * P],
                                 Wvo_s[:, j, :], start=(j == 0), stop=False)
            nc.tensor.matmul(out_ps[:], saT[:], bvoWo_row16[:], start=False,
                             stop=False)
            nc.tensor.matmul(out_ps[:], ones_row16[0:1, 0:P], bo_row16[:],
                             start=False, stop=True)
            out_sb = sml.tile([P, D], F32, tag="outsb")
            nc.vector.tensor_tensor(out_sb[:], out_ps[:], xt, ALU.add)
            nc.sync.dma_start(out_d[n0:n0 + P, :], out_sb[:])

    return nc


# ---------------------------------------------------------------------------
# Post-scheduling legalization: the walrus build here encodes at most ONE
# sync-wait command per TPB instruction (matmul LDWEIGHTS / CTRL structs
# reject more). Hoist excess waits onto same-engine EventSemaphore helpers
# inserted immediately before the offending instruction (sequencer order
# preserves blocking semantics exactly).
_LGL_UID = [0]


def legalize_waits(nc, cap=1):
    n_helpers = 0
    for fn in nc.m.functions:
        for bb in fn.blocks:
            out = []
            for ins in bb.instructions:
                si = ins.sync_info
                waits = list(si.on_wait) if si is not None else []
                if len(waits) > cap:
                    excess, keep = waits[:-cap], waits[-cap:]
                    for w in excess:
                        _LGL_UID[0] += 1
                        helper = mybir.InstEventSemaphore(
                            name=f"I-lgl-{_LGL_UID[0]}", ins=[], outs=[])
                        helper.engine = ins.engine
                        helper.sync_info = mybir.SyncInfo(
                            on_wait=[w], on_update=[])
                        out.append(helper)
                        n_helpers += 1
                    ins.sync_info = mybir.SyncInfo(
                        on_wait=keep,
                        on_update=list(si.on_update) if si is not None else [])
                out.append(ins)
            bb.instructions = out
    return n_helpers


_NC_CACHE = {}


def _get_nc(schedule, rows_per_core=ROWS_PER_CORE):
    key = (tuple(schedule), rows_per_core)
    if key not in _NC_CACHE:
        nc = build_nc(schedule, rows_per_core)
        legalize_waits(nc)
        _NC_CACHE[key] = nc
    return _NC_CACHE[key]


def prepare(inputs, rows_per_core=ROWS_PER_CORE, ncores=NCORES):
    """Host marshalling: sort + compact + schedule.

    Returns (in_maps, schedules, inv_orders).  All cores share ONE compiled
    kernel, so the schedule is the per-tile MAX slot count across cores.
    """
    import ml_dtypes

    q = np.ascontiguousarray(np.asarray(inputs["queries"], np.float32))
    feat = np.ascontiguousarray(
        np.asarray(inputs["image_features"], np.float32))
    pc = np.ascontiguousarray(np.asarray(inputs["pixel_coords"], np.float32))
    vm = np.ascontiguousarray(np.asarray(inputs["valid_mask"], np.int32))
    wshared = {
        "Wq": np.ascontiguousarray(np.asarray(inputs["Wq"], np.float32)),
        "bq": np.ascontiguousarray(np.asarray(inputs["bq"], np.float32)),
        "Wkv": np.ascontiguousarray(np.asarray(inputs["Wkv"], np.float32)),
        "bkv": np.ascontiguousarray(np.asarray(inputs["bkv"], np.float32)),
        "Wo": np.ascontiguousarray(np.asarray(inputs["Wo"], np.float32)),
        "bo": np.ascontiguousarray(np.asarray(inputs["bo"], np.float32)),
        "gamma": np.ascontiguousarray(np.asarray(inputs["gamma"], np.float32)),
        "beta": np.ascontiguousarray(np.asarray(inputs["beta"], np.float32)),
    }

    # --- 2x2 overlapping pixel blocks in bf16: (B, C*H*W, 4*D) ---
    fp = np.pad(feat, ((0, 0), (0, 0), (0, 1), (0, 1), (0, 0)), mode="edge")
    win = np.lib.stride_tricks.sliding_window_view(fp, (2, 2), axis=(2, 3))
    blk = win.transpose(0, 1, 2, 3, 5, 6, 4).astype(
        ml_dtypes.bfloat16).reshape(B, C * H * W, BLK)

    # --- bilinear offsets + weights (query-major, camera axis last) ---
    p = (pc + np.float32(1.0)) * np.float32(31.5)        # (B, C, N, 2) f32
    p0 = np.minimum(np.floor(p), np.float32(W - 2))
    fr = p - p0
    x0 = p0[..., 0].astype(np.int32)
    y0 = p0[..., 1].astype(np.int32)
    fx = fr[..., 0]
    fy = fr[..., 1]
    cam = (np.arange(C, dtype=np.int32) * (H * W))[None, :, None]
    offs = (cam + y0 * W + x0).astype(np.int32)          # (B, C, N)
    w_ab = np.stack([(1 - fy) * (1 - fx), (1 - fy) * fx,
                     fy * (1 - fx), fy * fx], axis=1)    # (B, 4, C, N)

    ntiles = rows_per_core // P
    per_b = N // (ncores // B)
    in_maps = []
    inv_orders = []
    Kmat = np.zeros((ncores, ntiles), np.int32)
    core_data = []
    for core in range(ncores):
        b = core // (ncores // B)
        n0 = (core % (ncores // B)) * per_b
        sl = slice(n0, n0 + rows_per_core)
        v = vm[b, :, sl]                          # (C, rows)
        kcnt = v.sum(0)                           # (rows,)
        order = np.argsort(kcnt, kind="stable")
        inv_orders.append(np.argsort(order))
        vs = v[:, order].T                        # (rows, C) sorted queries
        # valid cameras first (stable -> ascending cam id among valid)
        slot_cam = np.argsort(-vs, axis=1, kind="stable")  # (rows, C)
        take = lambda a: np.take_along_axis(a, slot_cam, axis=1)
        offs_s = take(offs[b, :, sl].T[order])             # (rows, C)
        valid_s = take(vs).astype(np.float32)              # (rows, C)
        w_s = np.stack([take(w_ab[b, ab, :, sl].T[order])
                        for ab in range(4)], axis=1)       # (rows, 4, C)
        w_s *= valid_s[:, None, :]
        offs_s = offs_s * (valid_s != 0)                   # padding -> block 0
        kq = vs.sum(1)                                     # sorted counts
        Kmat[core] = np.maximum(
            kq.reshape(ntiles, P).max(axis=1), 1)
        xq = q[b, sl][order]
        core_data.append((xq, valid_s, offs_s.astype(np.int32),
                          w_s.reshape(rows_per_core, 4 * C), b))
    schedule = Kmat.max(axis=0)

    import ml_dtypes as mld
    for xq, valid_s, offs_s, w_s, b in core_data:
        m = {
            "xv": np.ascontiguousarray(
                np.concatenate([xq, valid_s], axis=-1)),
            "blk": blk[b],
            "offs": np.ascontiguousarray(offs_s),
            "w4": np.ascontiguousarray(w_s.astype(mld.bfloat16)),
        }
        m.update(wshared)
        in_maps.append(m)
    return in_maps, schedule, inv_orders


def kernel(**inputs) -> np.ndarray:
    from concourse.bass_utils import run_bass_kernel_spmd
    in_maps, schedule, inv_orders = prepare(inputs)
    nc = _get_nc(schedule)
    res = run_bass_kernel_spmd(nc, in_maps, core_ids=list(range(NCORES)))
    outs = [np.asarray(r["out"])[inv_orders[c]]
            for c, r in enumerate(res.results)]
    full = np.concatenate(outs, axis=0).reshape(B, N, D)
    return full.astype(np.float32)
